# revision 1
# baseline (speedup 1.0000x reference)
"""Trainium2 Bass kernel for CombinedLoss (chamfer + density) on 8 NeuronCores.

Problem: B=4, N=M=8192, D=3.
  chamfer = mean_n min_m d2[b,n,m] + mean_m min_n d2[b,n,m],  d2 clamped >= 0
  density = mean |pred_densities|
  total   = chamfer_clipped + 0.1 * density

Strategy (self-contained, hardcoded shapes):
  - 8 cores = 4 batches x 2 halves. Core c handles batch c//2, point-half c%2.
  - Two passes per core, each pass computes free-axis row-mins of the distance
    matrix with the "row" points as the matmul stationary operand:
      pass A: rows = 4096 pred points,  cols = all 8192 target points -> cham_x
      pass B: rows = 4096 target points, cols = all 8192 pred points  -> cham_y
  - d2 is produced directly in PSUM by one K=24 matmul: coordinates and squared
    norms are split host-side into exact bf16 triples (x = xh+xm+xl captures all
    24 f32 mantissa bits), and the 24 contraction rows are ordered largest-first
    so f32 PSUM accumulation cancels early. Result is f32-accurate d2.
  - Per row tile (128 rows x 8192 cols = 4 PSUM groups of [128,2048]): VectorE
    min-reduces one group straight from PSUM while ScalarE evacuates the other
    three to fp16 SBUF; VectorE then chains them with 2x-rate fp16
    tensor_tensor mins and one short reduce. Both reduce-capable engines run
    near-saturated; TensorE (~218us) hides underneath.
  - Host: clamp mins at 0, means in f64, assemble the three scalars.
"""

import os
from contextlib import ExitStack

import ml_dtypes
import numpy as np

import concourse.tile as tile
from concourse import bacc, mybir
from concourse.bass_utils import run_bass_kernel_spmd

B, N, M, D = 4, 8192, 8192, 3
R = N // 2          # rows per core per pass
NT = R // 128       # 32 row tiles
NG = M // 2048      # 4 column groups
K = 24              # contraction rows of the distance matmul

BF16 = ml_dtypes.bfloat16

# "tree2b6" (default): per row tile, 1 column group is min-reduced by VectorE
# directly from PSUM while ScalarE evacuates the other 3 to fp16 SBUF for a
# VectorE 2x-rate tensor_tensor min chain; 6-slot pools for pipelining.
# "simple": plain VectorE tensor_reduce over PSUM (slow, few moving parts).
MODE = os.environ.get("CHAMFER_MODE", "tree2b6dsp")


def _split3(a_f64):
    """Split values into 3 bf16 parts summing (near-)exactly to the input."""
    p0 = a_f64.astype(BF16)
    r1 = a_f64 - p0.astype(np.float64)
    p1 = r1.astype(BF16)
    r2 = r1 - p1.astype(np.float64)
    p2 = r2.astype(BF16)
    return p0, p1, p2


def _build_operands(rows_pts, cols_pts):
    """Stationary [K, R] and moving [K, ncols] bf16 matrices so that
    (stat.T @ mov)[i, j] = ||rows_pts[i] - cols_pts[j]||^2 in f32-grade accuracy.

    rows_pts: [R, 3] f32; cols_pts: [ncols, 3] f32.
    """
    a = rows_pts.astype(np.float64)
    b = cols_pts.astype(np.float64)
    a2 = (a * a).sum(-1)
    b2 = (b * b).sum(-1)
    ah, am, al = _split3(a.T)      # each [3, R]
    bh, bm, bl = _split3(b.T)      # each [3, ncols]
    a2h, a2m, a2l = _split3(a2)    # [R]
    b2h, b2m, b2l = _split3(b2)    # [ncols]

    nr, ncols = a.shape[0], b.shape[0]
    S = np.zeros((K, nr), BF16)
    Mv = np.zeros((K, ncols), BF16)
    ones_r = np.ones((nr,), BF16)
    ones_c = np.ones((ncols,), BF16)

    def neg2(t):
        return (-2.0 * t.astype(np.float32)).astype(BF16)  # exact for bf16 input

    # rows ordered largest magnitude first for benign psum accumulation order
    S[0], Mv[0] = a2h, ones_c
    S[1], Mv[1] = ones_r, b2h
    S[2:5], Mv[2:5] = neg2(ah), bh          # hh
    S[5], Mv[5] = a2m, ones_c
    S[6], Mv[6] = ones_r, b2m
    S[7:10], Mv[7:10] = neg2(ah), bm        # hm
    S[10:13], Mv[10:13] = neg2(am), bh      # mh
    S[13], Mv[13] = a2l, ones_c
    S[14], Mv[14] = ones_r, b2l
    S[15:18], Mv[15:18] = neg2(ah), bl      # hl
    S[18:21], Mv[18:21] = neg2(al), bh      # lh
    S[21:24], Mv[21:24] = neg2(am), bm      # mm
    return S, Mv


def _emit_pass(ctx, tc, pools, stat_ap, mov_ap, out_ap, mode):
    nc = tc.nc
    big, psum, small = pools
    f32 = mybir.dt.float32
    bf16 = mybir.dt.bfloat16
    fp16 = mybir.dt.float16
    MIN = mybir.AluOpType.min

    mov_sb = big.tile([K, M], bf16, tag="mov")
    stat_sb = big.tile([K, R], bf16, tag="stat")
    if "dsp" in mode:
        # split input DMAs into column chunks so several queues run in parallel
        nm = 8 if "dsp8" in mode else 4
        ns = 4 if "dsp8" in mode else 2
        for c in range(nm):
            nc.sync.dma_start(
                mov_sb[:, c * (M // nm) : (c + 1) * (M // nm)],
                mov_ap[:, c * (M // nm) : (c + 1) * (M // nm)],
            )
        for c in range(ns):
            nc.sync.dma_start(
                stat_sb[:, c * (R // ns) : (c + 1) * (R // ns)],
                stat_ap[:, c * (R // ns) : (c + 1) * (R // ns)],
            )
    else:
        nc.sync.dma_start(mov_sb[:], mov_ap[:])
        nc.sync.dma_start(stat_sb[:], stat_ap[:])
    rowred = big.tile([128, NT], f32, tag="rowred")

    mm_w = 1024 if "wide" in mode else 512
    n_mm = 2048 // mm_w

    def fill(ps, t, g):
        for s in range(n_mm):
            nc.tensor.matmul(
                ps[:, mm_w * s : mm_w * (s + 1)],
                lhsT=stat_sb[:, 128 * t : 128 * (t + 1)],
                rhs=mov_sb[:, 2048 * g + mm_w * s : 2048 * g + mm_w * (s + 1)],
                start=True,
                stop=True,
            )

    X = mybir.AxisListType.X

    if mode == "simple":
        for t in range(NT):
            red4 = small.tile([128, NG], f32, tag="red4")
            for g in range(NG):
                ps = psum.tile([128, 2048], f32, tag="ps")
                fill(ps, t, g)
                nc.vector.tensor_reduce(red4[:, g : g + 1], ps[:], axis=X, op=MIN)
            nc.vector.tensor_reduce(rowred[:, t : t + 1], red4[:], axis=X, op=MIN)
        nc.sync.dma_start(out_ap[:], rowred[:])
        return

    # "z" modes fill the direct group last so ScalarE copies start earlier
    def gorder(n_direct):
        if "z" in mode:
            return list(range(n_direct, NG)) + list(range(n_direct))
        return list(range(NG))

    # per-tile direct-path ("d") and copied-path ("c") group counts
    if mode.startswith("d2tree"):
        base_direct = 2
    elif mode.startswith("tree0"):
        base_direct = 0
    else:
        base_direct = 1
    # redall[:, j, t]: j=0 direct-path min, j=1 fp16-tree min, for row tile t
    redall = big.tile([128, 2, NT], f32, tag="redall")
    for t in range(NT):
        # fractional direct share: drop the direct group on some tiles to
        # shift reduce work from VectorE to ScalarE
        if "alt2" in mode:
            n_direct = base_direct if t % 2 == 0 else 0
        elif "alt4" in mode:
            n_direct = 0 if t % 4 == 3 else base_direct
        else:
            n_direct = base_direct
        order = gorder(n_direct)
        if n_direct == 0:
            direct_gs = []
        else:
            direct_gs = order[-n_direct:] if "z" in mode else order[:n_direct]
        copied_gs = [g for g in order if g not in direct_gs]
        # direct groups: VectorE min-reduce straight from PSUM
        rd2 = None
        emitted = []

        def emit_direct(d, g):
            ps = psum.tile([128, 2048], f32, tag="ps")
            fill(ps, t, g)
            nonlocal rd2
            if d == 0:
                dst = redall[:, 0, t : t + 1]
            else:
                rd2 = small.tile([128, 1], f32, tag="rd2")
                dst = rd2[:, 0:1]
            nc.vector.tensor_reduce(dst, ps[:], axis=X, op=MIN)

        # copied groups: ScalarE evacuates to fp16 SBUF; VectorE mins them
        # with 2x-rate fp16 tensor_tensor ops and one short 1x reduce.
        cps = []

        def emit_copied(i, g):
            ps = psum.tile([128, 2048], f32, tag="ps")
            fill(ps, t, g)
            cp = small.tile([128, 2048], fp16, tag=f"cp{i}")
            nc.scalar.copy(cp[:], ps[:])
            cps.append(cp)

        di = ci = 0
        for g in order:
            if g in direct_gs:
                emit_direct(di, g)
                di += 1
            else:
                emit_copied(ci, g)
                ci += 1
        h1 = small.tile([128, 2048], fp16, tag="h1")
        nc.vector.tensor_tensor(h1[:], cps[0][:], cps[1][:], op=MIN)
        h = h1
        for extra in cps[2:]:
            h2 = small.tile([128, 2048], fp16, tag="h2")
            nc.vector.tensor_tensor(h2[:], h[:], extra[:], op=MIN)
            h = h2
        if mode.startswith("tree2a"):
            h3 = small.tile([128, 1024], fp16, tag="h3")
            nc.vector.tensor_tensor(h3[:], h[:, 0:1024], h[:, 1024:2048], op=MIN)
            h = h3
        if ("alt2" in mode or "alt4" in mode) and n_direct == 0:
            nc.vector.tensor_reduce(redall[:, 1, t : t + 1], h[:], axis=X, op=MIN)
            nc.scalar.copy(redall[:, 0, t : t + 1], redall[:, 1, t : t + 1])
        else:
            tree_dst = redall[:, 1 if n_direct else 0, t : t + 1]
            nc.vector.tensor_reduce(tree_dst, h[:], axis=X, op=MIN)
        if n_direct == 2:
            nc.vector.tensor_tensor(
                redall[:, 0, t : t + 1], redall[:, 0, t : t + 1], rd2[:, 0:1], op=MIN
            )
    if n_direct:
        nc.vector.tensor_tensor(rowred[:], redall[:, 0, :], redall[:, 1, :], op=MIN)
        nc.sync.dma_start(out_ap[:], rowred[:])
    else:
        nc.sync.dma_start(out_ap[:], redall[:, 0, :])


def _build_program(rep: int = 1, mode: str | None = None):
    mode = MODE if mode is None else mode
    nc = bacc.Bacc("TRN2", target_bir_lowering=False, debug=False, num_devices=8)
    bf16 = mybir.dt.bfloat16
    f32 = mybir.dt.float32
    statA = nc.dram_tensor("statA", [K, R], bf16, kind="ExternalInput").ap()
    movA = nc.dram_tensor("movA", [K, M], bf16, kind="ExternalInput").ap()
    statB = nc.dram_tensor("statB", [K, R], bf16, kind="ExternalInput").ap()
    movB = nc.dram_tensor("movB", [K, M], bf16, kind="ExternalInput").ap()
    rowA = nc.dram_tensor("rowA", [128, NT], f32, kind="ExternalOutput").ap()
    rowB = nc.dram_tensor("rowB", [128, NT], f32, kind="ExternalOutput").ap()

    with tile.TileContext(nc) as tc:
        with ExitStack() as ctx:
            big = ctx.enter_context(tc.tile_pool(name="big", bufs=2))
            psum = ctx.enter_context(tc.tile_pool(name="psum", bufs=2, space="PSUM"))
            nbufs = 6 if "b6" in mode else 4
            small = ctx.enter_context(tc.tile_pool(name="small", bufs=nbufs))
            pools = (big, psum, small)

            def body(_i=None):
                _emit_pass(ctx, tc, pools, statA, movA, rowA, mode)
                _emit_pass(ctx, tc, pools, statB, movB, rowB, mode)

            if rep == 1:
                body()
            else:
                with tc.For_i(0, rep, 1) as i:
                    body(i)
    nc.compile()
    return nc


_NC_CACHE = None


def _get_program():
    global _NC_CACHE
    if _NC_CACHE is None:
        _NC_CACHE = _build_program()
    return _NC_CACHE


def _decode_rowmin(arr):
    # arr [128, NT] with value for local row t*128+p at [p, t]
    return arr.T.reshape(R)


def _make_in_maps(pred_points, target_points):
    in_maps = []
    for c in range(8):
        b, h = divmod(c, 2)
        x_half = pred_points[b, h * R : (h + 1) * R]
        y_half = target_points[b, h * R : (h + 1) * R]
        SA, MA = _build_operands(x_half, target_points[b])
        SB, MB = _build_operands(y_half, pred_points[b])
        in_maps.append({"statA": SA, "movA": MA, "statB": SB, "movB": MB})
    return in_maps


def kernel(pred_points, target_points, pred_densities):
    pred_points = np.asarray(pred_points, np.float32)
    target_points = np.asarray(target_points, np.float32)
    pred_densities = np.asarray(pred_densities, np.float32)

    nc = _get_program()
    in_maps = _make_in_maps(pred_points, target_points)
    res = run_bass_kernel_spmd(nc, in_maps, core_ids=list(range(8)))

    mins_x = np.empty((B, N), np.float64)
    mins_y = np.empty((B, M), np.float64)
    for c in range(8):
        b, h = divmod(c, 2)
        mins_x[b, h * R : (h + 1) * R] = _decode_rowmin(res.results[c]["rowA"])
        mins_y[b, h * R : (h + 1) * R] = _decode_rowmin(res.results[c]["rowB"])

    cham_x = np.maximum(mins_x, 0.0).mean()
    cham_y = np.maximum(mins_y, 0.0).mean()
    chamfer = np.clip(cham_x + cham_y, 0.0, 1.0e6)
    density = np.abs(pred_densities.astype(np.float64)).mean()
    total = 1.0 * chamfer + 0.1 * density
    return (
        np.float32(total),
        np.float32(chamfer),
        np.float32(density),
    )

